# revision 78
# baseline (speedup 1.0000x reference)
"""Trainium2 Bass kernel for nn_Decoder (worker/task label-probability decoder).

Math:
    worker_feature = inputs[:2048, :64]          # [Wn, A]
    tau            = inputs[2048:, :16]          # [T, L]
    x  = worker_feature @ W + b                  # [Wn]
    p1 = sigmoid(x);  p2 = (1 - p1) / (L - 1)
    P[i, j, l] = p1[i]^tau[j,l] * p2[i]^(1 - tau[j,l])
               = exp(a[i] * tau[j,l] + c[i])
    with a = ln p1 - ln p2 = x + ln(L-1),  c = ln p2 = -ln(1 + e^x) - ln(L-1)

Sharding: pure data parallel over the worker axis (dim 0), 256 workers per
core across 8 cores; tau/W/b replicated. No communication.

Per-core device layout: workers on SBUF partitions (2 groups of 128), task
axis flattened on the free dimension. tau (fp32, exact) is replicated to all
128 partitions by stride-0 broadcast DMA loads into rotating SBUF stripes,
issued on the otherwise idle SP and GPSIMD engines. The scalar engine
computes Exp(a*tau + c) with per-partition scale/bias over ~84% of the
columns, writing bf16 directly (the 2e-2 accuracy budget dwarfs bf16's
2e-3 rounding); the vector engine computes the rest with an exp2
bit-trick (magic-number round, bf16 quadratic for 2^r, integer exponent
add in bf16 bit space). Outputs stream to HBM interleaved on the two DMA
engines, and the host upcasts to fp32. The scalar engine runs stall-free
start to finish and is the critical path, with the vector engine finishing
just under it.
"""

import math

import numpy as np

try:
    import concourse.bass as bass  # noqa: F401
except ImportError:  # fall back to the container's repo checkout
    import sys

    for _p in ("/root/.axon_site/_ro/trn_rl_repo", "/opt/trn_rl_repo"):
        if _p not in sys.path:
            sys.path.append(_p)

import concourse.bass as bass
import concourse.tile as tile
from concourse import library_config, mybir
from concourse.bass_utils import run_bass_kernel_spmd
from concourse.vector_clock import ScopedClock

WN = 2048  # workers total
TN = 2048  # tasks
L = 16  # edge types / labels
A = 64  # ability features
NCORES = 8
WPC = WN // NCORES  # workers per core (256)
G = WPC // 128  # partition groups per core (2)
F = TN * L  # flattened task axis (32768)

_AF = mybir.ActivationFunctionType


class _TC(tile.TileContext):
    """TileContext legalized for a walrus that allows one sync-wait per inst.

    The walrus build in this container rejects any instruction carrying more
    than one sync-wait command. After Tile's normal scheduling + the exit
    drain/barrier, rewrite every multi-wait instruction into a chain of
    same-engine NOPs (one wait each) followed by the instruction with the
    final wait.
    """

    def _drain_and_barrier(self, tick_clock, wait_clock):
        super()._drain_and_barrier(tick_clock, wait_clock)
        self._split_multi_waits()

    def _fresh_nop(self, engine):
        inst = self.nc.engines[engine].nop(nofuse=True).ins
        self.nc.cur_bb.bb.instructions.remove(inst)
        return inst

    def _split_multi_waits(self):
        for fn in self.nc.m.functions:
            for bb in fn.blocks:
                snapshot = list(bb.instructions)
                if not any(
                    inst.sync_info and len(inst.sync_info.on_wait) > 1
                    for inst in snapshot
                ):
                    continue
                new = []
                for inst in snapshot:
                    si = inst.sync_info
                    if si is not None and si.on_wait and len(si.on_wait) > 1:
                        waits = list(si.on_wait)
                        si.on_wait = waits[-1:]
                        inst.sync_info = si
                        for wt in waits[:-1]:
                            nop = self._fresh_nop(inst.engine)
                            nop.sync_info = mybir.SyncInfo(on_wait=[wt], on_update=[])
                            new.append(nop)
                    new.append(inst)
                bb.instructions[:] = new


def build_nc():
    ln_lm1 = math.log(float(L - 1))
    nc = bass.Bass("TRN2")
    wfwb_in = nc.dram_tensor(
        "wfwb", [128, 3 * (A + 1)], mybir.dt.float32, kind="ExternalInput"
    )
    tauf_in = nc.dram_tensor("tauf", [F], mybir.dt.float32, kind="ExternalInput")
    out = nc.dram_tensor("out", [G, 128, F], mybir.dt.bfloat16, kind="ExternalOutput")

    f32 = mybir.dt.float32
    bf16 = mybir.dt.bfloat16

    with _TC(nc) as tc:
        # The scalar engine handles cols [0, ACT_F); the vector engine
        # computes the remaining DVE_F cols with a bit-trick exp2 (it has no
        # activation table, but ~9 us/slice of otherwise idle throughput
        # still relieves the scalar engine, the critical path).
        DVE_F = 4096
        ACT_F = F - DVE_F

        # Stripe schedule: (width, act sub-op widths, load piece widths).
        # First stripe is ragged so the first activation launches on a small
        # early piece; the last stripe is narrow to shorten the drain tail.
        SCHED = [
            (4096, [1024, 3072], [1024, 1024, 2048]),
            (4096, [4096], [2048] * 2),
            (4096, [4096], [2048] * 2),
            (4096, [4096], [2048] * 2),
            (4096, [4096], [2048] * 2),
            (4096, [4096], [2048] * 2),
            (4096, [2048, 2048], [2048] * 2),
        ]
        assert sum(w for w, _, _ in SCHED) == ACT_F

        with (
            tc.tile_pool(name="const", bufs=1) as const,
            tc.tile_pool(name="taus", bufs=2) as taus,
            tc.tile_pool(name="outs", bufs=2) as outs,
        ):
            # Warm the exp/ln activation table before any input lands so the
            # prep chain doesn't pay the table load on the critical path.
            warm = const.tile([128, 1], f32)
            warm_out = const.tile([128, 1], f32)
            nc.vector.memset(warm, 1.0)
            nc.scalar.activation(warm_out, warm, _AF.Exp)
            # warm stays 1.0 and doubles as the +1 bias for ln(e^x + 1) below

            # ---- tau stripe loads: stride-0 broadcast straight from HBM ----
            # The DMA cost model charges per-partition free bytes on the
            # issuing engine, so the ~50 us of broadcast loads and ~50 us of
            # output stores are split between SP and GPSIMD, both hidden under
            # the scalar engine's exp stream.
            eng = [nc.sync, nc.gpsimd]
            tau_ap = tauf_in[:]
            stripes = {}
            bases = []
            b0 = 0
            for w, _, _ in SCHED:
                bases.append(b0)
                b0 += w

            def load_stripe(s, engines=None):
                w, _, pieces = SCHED[s]
                st = taus.tile([128, w], f32, tag="st", name=f"st{s}", bufs=3)
                stripes[s] = st
                off = 0
                for i, pw in enumerate(pieces):
                    e = eng[i % 2] if engines is None else engines[i]
                    e.dma_start(
                        out=st[:, off : off + pw],
                        in_=bass.AP(
                            tensor=tau_ap.tensor,
                            offset=tau_ap.offset + bases[s] + off,
                            ap=[[0, 128], [1, pw]],
                        ),
                    )
                    off += pw

            # Prologue: one host-packed DMA carries worker features (with a
            # constant-1 column so the bias rides along) plus Wb = [W; b]
            # broadcast: per partition [wf_g0, 1, wf_g1, 1, Wb].
            wfwb = const.tile([128, 3, A + 1], f32)
            nc.sync.dma_start(out=wfwb, in_=wfwb_in[:])
            # tau stripe for the vector engine's slices: cols [ACT_F, F).
            # A small lead piece arrives early so the DVE starts by ~6 us.
            std = taus.tile([128, DVE_F], f32, tag="stD", name="stD", bufs=1)
            DVE_PIECES = [(0, 1024, 0), (1024, 1024, 1), (2048, 2048, 0)]

            def load_dve_piece(i):
                o, pw, e = DVE_PIECES[i]
                eng[e].dma_start(
                    out=std[:, o : o + pw],
                    in_=bass.AP(
                        tensor=tau_ap.tensor,
                        offset=tau_ap.offset + ACT_F + o,
                        ap=[[0, 128], [1, pw]],
                    ),
                )

            # Stripe 0 pieces interleaved with the DVE lead piece on SP so
            # the vector engine starts by ~4.3 us; ACT's second piece still
            # lands before its consumer.
            w0, _, pieces0 = SCHED[0]
            st0 = taus.tile([128, w0], f32, tag="st", name="st0", bufs=3)
            stripes[0] = st0

            def _p0(e, off, pw):
                e.dma_start(
                    out=st0[:, off : off + pw],
                    in_=bass.AP(
                        tensor=tau_ap.tensor,
                        offset=tau_ap.offset + off,
                        ap=[[0, 128], [1, pw]],
                    ),
                )

            _p0(nc.sync, 0, 1024)
            load_dve_piece(0)
            _p0(nc.sync, 1024, 1024)
            _p0(nc.gpsimd, 2048, 2048)
            load_dve_piece(1)
            load_stripe(1)

            # ---- per-worker scalars ----
            x = const.tile([128, G], f32)
            prod = const.tile([128, G, A + 1], f32)
            wb_row = wfwb[:, 2, :]
            nc.vector.tensor_mul(
                prod,
                wfwb[:, :G, :],
                bass.AP(
                    tensor=wb_row.tensor,
                    offset=wb_row.offset,
                    ap=[wb_row.ap[0], [0, G], wb_row.ap[1]],
                ),
            )
            nc.vector.reduce_sum(x, prod, axis=mybir.AxisListType.X)

            a_sb = const.tile([128, G], f32)
            nc.vector.tensor_scalar_add(a_sb, x, ln_lm1)
            ex = const.tile([128, G], f32)
            nc.scalar.activation(ex, x, _AF.Exp)
            lse = const.tile([128, G], f32)
            nc.scalar.activation(lse, ex, _AF.Ln, bias=warm[:, 0:1])  # ln(e^x + 1)
            # lp2 = -lse - ln(L-1) stays on the scalar engine (Identity is in
            # the same activation-table set), avoiding a cross-engine hop on
            # the prep critical path.
            neg_ln = const.tile([128, 1], f32)
            nc.vector.memset(neg_ln, -ln_lm1)
            lp2 = const.tile([128, G], f32)
            nc.scalar.activation(
                lp2, lse, _AF.Identity, bias=neg_ln[:, 0:1], scale=-1.0
            )

            # Per-worker constants for the DVE exp2 bit-trick:
            #   t   = a2*tau + cl    (= log2 of the result)
            #   u   = t + MAGIC      (fp32 add rounds t to integer n)
            #   rn  = (u - MAGIC) - t  (= n - t in [-.5, .5], stored bf16)
            #   q   = c0 + rn*(-c1 + c2*rn)  (bf16 quadratic for 2^(-rn))
            #   m   = (u*128 - MAGIC*128) -> int16  (= n << 7)
            #   out = bits(q) + m  (int16 add == bf16 exponent adjust)
            LOG2E = 1.4426950408889634
            MAGIC = 12582912.0  # 1.5 * 2^23
            a2 = const.tile([128, G], f32)
            nc.vector.tensor_scalar_mul(a2, a_sb, LOG2E)
            cl = const.tile([128, G], f32)
            nc.vector.tensor_scalar_mul(cl, lp2, LOG2E)

            # ---- DVE exp2 chains (cols [ACT_F, F)), emitted interleaved ----
            i16 = mybir.dt.int16
            C0, C1, C2 = 1.00035163, 0.70128093, 0.23734974
            DPC = 2048  # max dve slice width (intermediate tile size)
            # (source, column offset in source, width, group, out column)
            dve_slices = [
                (None, o, w, g, o)
                for o, w in ((0, 1024), (1024, 1024), (2048, 2048))
                for g in range(G)
            ] + [(6, 2048, 2048, 0, None)]

            def emit_dve_slice(i):
                if i >= len(dve_slices):
                    return
                src, off, dw, g, ocol = dve_slices[i]
                v = nc.vector
                tt = const.tile([128, DPC], f32, tag="dv_tt")
                u = const.tile([128, DPC], f32, tag="dv_u")
                r16 = const.tile([128, DPC], bf16, tag="dv_r")
                w16 = const.tile([128, DPC], bf16, tag="dv_w")
                t16 = const.tile([128, DPC], bf16, tag="dv_t")
                q16 = const.tile([128, DPC], bf16, tag="dv_q")
                m16 = const.tile([128, DPC], i16, tag="dv_m")
                ob = outs.tile(
                    [128, DPC], bf16, tag="dv_o", name=f"dv_o{i}", bufs=2
                )
                if src is None:
                    st_in = std[:, off : off + dw]
                    obase = ACT_F + ocol
                else:
                    st_in = stripes[src][:, off : off + dw]
                    obase = bases[src] + off
                v.tensor_scalar(
                    tt[:, :dw], st_in, a2[:, g : g + 1], cl[:, g : g + 1],
                    mybir.AluOpType.mult, mybir.AluOpType.add,
                )
                v.tensor_scalar(u[:, :dw], tt[:, :dw], MAGIC, None, mybir.AluOpType.add)
                # rn = (u - MAGIC) - t = n - t  (exact; in [-.5, .5])
                v.scalar_tensor_tensor(
                    r16[:, :dw], u[:, :dw], MAGIC, tt[:, :dw],
                    mybir.AluOpType.subtract, mybir.AluOpType.subtract,
                )
                # q = 2^(t - n) = c0 - c1*rn + c2*rn^2
                v.tensor_scalar(
                    w16[:, :dw], r16[:, :dw], C2, -C1,
                    mybir.AluOpType.mult, mybir.AluOpType.add,
                )
                v.tensor_tensor(t16[:, :dw], w16[:, :dw], r16[:, :dw], mybir.AluOpType.mult)
                v.tensor_scalar(q16[:, :dw], t16[:, :dw], C0, None, mybir.AluOpType.add)
                v.tensor_scalar(
                    m16[:, :dw], u[:, :dw], 128.0, MAGIC * 128.0,
                    mybir.AluOpType.mult, mybir.AluOpType.subtract,
                )
                v.tensor_tensor(
                    ob.bitcast(i16)[:, :dw], q16.bitcast(i16)[:, :dw], m16[:, :dw],
                    mybir.AluOpType.add,
                )
                nonlocal store
                e = nc.gpsimd if src is not None else eng[store % 2]
                e.dma_start(out=out[g, :, obase : obase + dw], in_=ob[:, :dw])
                store += 1

            # ---- main loop: ACT exp over SBUF stripes -> bf16 -> DMA out ----
            store = 1
            for s in range(len(SCHED)):
                st = stripes[s]
                _, subs, _ = SCHED[s]
                off = 0
                last = s == len(SCHED) - 1
                for fd in subs:
                    for g in range(G):
                        if s == 6 and off == 2048 and g == 0:
                            continue  # this slice is computed on the DVE
                        ot = outs.tile(
                            [128, 2048 if last else 4096],
                            bf16,
                            tag=f"ot{g}_tail" if last else f"ot{g}",
                            name=f"ot{g}_{s}_{off}",
                            bufs=3,
                        )
                        nc.scalar.activation(
                            ot[:, :fd],
                            st[:, off : off + fd],
                            _AF.Exp,
                            bias=lp2[:, g : g + 1],
                            scale=a_sb[:, g : g + 1],
                        )
                        for so in range(0, fd, 4096):
                            sw = min(4096, fd - so)
                            if last and off == 2048 and g == 1:
                                # final slice: the scalar engine is idle once
                                # its stream ends, so its empty HWDGE ring
                                # issues this store with zero queueing delay.
                                # Tiny dummy DMAs first absorb the congested
                                # completion-sem lane so this store rotates to
                                # one with an earlier predecessor.
                                for _d in range(2):
                                    dmy = const.tile([128, 1], f32, tag=f"dmy{_d}")
                                    nc.scalar.dma_start(
                                        out=dmy,
                                        in_=bass.AP(
                                            tensor=tau_ap.tensor,
                                            offset=tau_ap.offset,
                                            ap=[[0, 128], [1, 1]],
                                        ),
                                    )
                                e = nc.scalar
                            else:
                                e = eng[store % 2]
                            e.dma_start(
                                out=out[
                                    g,
                                    :,
                                    bases[s] + off + so : bases[s] + off + so + sw,
                                ],
                                in_=ot[:, so : so + sw],
                            )
                            store += 1
                    off += fd
                emit_dve_slice(s)
                if s + 2 < len(SCHED):
                    load_stripe(s + 2)
                if s == 1:
                    load_dve_piece(2)
    return nc


_NC = None


def kernel(inputs, W, b, worker_num=WN, task_num=TN, edge_type=L, ability_num=A, **_kw):
    global _NC
    inputs = np.ascontiguousarray(np.asarray(inputs, dtype=np.float32))
    W = np.asarray(W, dtype=np.float32).reshape(A)
    b = np.asarray(b, dtype=np.float32).reshape(1)
    assert inputs.shape == (WN + TN, A)

    wf = inputs[:WN, :A]
    tauf = np.ascontiguousarray(inputs[WN:, :L].reshape(F))
    wb = np.concatenate([W, b]).astype(np.float32)

    if _NC is None:
        _NC = build_nc()

    def pack_wfwb(k):
        blk = np.empty((128, 3, A + 1), dtype=np.float32)
        shard = wf[k * WPC : (k + 1) * WPC]
        for g in range(G):
            blk[:, g, :A] = shard[g * 128 : (g + 1) * 128]
            blk[:, g, A] = 1.0
        blk[:, 2, :] = wb
        return np.ascontiguousarray(blk.reshape(128, 3 * (A + 1)))

    in_maps = [{"wfwb": pack_wfwb(k), "tauf": tauf} for k in range(NCORES)]
    res = run_bass_kernel_spmd(_NC, in_maps, core_ids=list(range(NCORES)))
    parts = [
        np.asarray(r["out"]).astype(np.float32).reshape(WPC, TN, L)
        for r in res.results
    ]
    return np.concatenate(parts, axis=0)
